# revision 1
# baseline (speedup 1.0000x reference)
"""ConvexPolytopeManifold expmap kernel for 8 Trainium2 NeuronCores.

Algorithm (matches reference.py):
    Q = A @ A.T
    z = projx(x+u):  50 its of lam <- relu(lam - step*(lam@Q - c)), c = (x+u)@A.T - b
    out = proju(z,u): active = (z@A.T >= b - tol); masked = (u@A.T)*active
                      10 its of lam <- relu(lam - step*(lam@Q - masked))*active
                      out = u - lam@A

Numerics: the PGD loops run in *delta form* — y (pre-relu state) and lam are
kept in fp32 in SBUF; only the per-iteration increment d = relu(y)-lam goes
through the PE at float32r (round-to-nearest-11-bit-mantissa operands, fp32
accumulate), and its bf16-class error is damped by step=0.01:
    y <- y + d - step*(Q_r @ d)
The d tile is written by the DVE *as f32r* (rounds on write), so the PE, the
lam accumulation and the y accumulation all consume the identical value —
the recursion stays exactly consistent with lam = sum(d).
All one-shot matmuls on the mask-critical path (c, z, z@A.T, u@A.T, out)
run in plain fp32 (4 cyc/row) for exactness.

Sharding: data-parallel over batch B=4096 -> 8 cores x 512 rows; A, b, Q
replicated per core. No cross-core communication.
"""
import numpy as np
from contextlib import ExitStack

import concourse.bass as bass
import concourse.tile as tile
from concourse import bacc, mybir
from concourse.bass_utils import run_bass_kernel_spmd
from concourse.masks import make_identity

dt = mybir.dt
F32, F32R, F16, BF16 = dt.float32, dt.float32r, dt.float16, dt.bfloat16
Alu = mybir.AluOpType

B, NF, M = 4096, 512, 1024      # batch, n features, m constraints
NCORES = 8
BPC = B // NCORES               # 512 batch rows per core
PROJ_ITERS, PROJU_ITERS = 50, 10
STEP, TOL = 0.01, 1e-5
MC = M // 128                   # 8 m-chunks
NC_ = NF // 128                 # 4 n-chunks
BC = BPC // 128                 # 4 batch-chunks

_cache = {}
_REPS = 1   # bench hook: >1 wraps the whole per-core program in For_i
LOOP_DT = F32R  # PGD loop matmul dtype: F32R or F16


def _build():
    nc = bacc.Bacc("TRN2", target_bir_lowering=False, debug=False,
                   num_devices=NCORES)
    xd = nc.dram_tensor("x", [BPC, NF], F32, kind="ExternalInput").ap()
    ud = nc.dram_tensor("u", [BPC, NF], F32, kind="ExternalInput").ap()
    Ad = nc.dram_tensor("A", [M, NF], F32, kind="ExternalInput").ap()
    bd = nc.dram_tensor("b", [M, 1], F32, kind="ExternalInput").ap()
    od = nc.dram_tensor("out", [BPC, NF], F32, kind="ExternalOutput").ap()

    import contextlib
    with tile.TileContext(nc) as tc, ExitStack() as ctx:
        pool = ctx.enter_context(tc.tile_pool(name="main", bufs=1))
        dpool = ctx.enter_context(tc.tile_pool(name="dbuf", bufs=2))
        psum = ctx.enter_context(tc.tile_pool(name="ps", bufs=8, space="PSUM"))

        rep_loop = tc.For_i(0, _REPS) if _REPS > 1 else contextlib.nullcontext()
        ctx.enter_context(rep_loop)

        # ---------- loads ----------
        x4 = []   # becomes w = x+u (in place)
        u4 = []
        A8 = []
        bc8 = []
        for i in range(BC):
            t = pool.tile([128, NF], F32, tag=f"x{i}")
            nc.sync.dma_start(t[:], xd[i*128:(i+1)*128, :]); x4.append(t)
            t = pool.tile([128, NF], F32, tag=f"u{i}")
            nc.sync.dma_start(t[:], ud[i*128:(i+1)*128, :]); u4.append(t)
        for m in range(MC):
            t = pool.tile([128, NF], F32, tag=f"A{m}")
            nc.sync.dma_start(t[:], Ad[m*128:(m+1)*128, :]); A8.append(t)
            t = pool.tile([128, 1], F32, tag=f"b{m}")
            nc.sync.dma_start(t[:], bd[m*128:(m+1)*128, :]); bc8.append(t)

        ident = pool.tile([128, 128], F32, tag="ident")
        make_identity(nc, ident[:])

        # w = x + u  (into x tiles)
        for i in range(BC):
            nc.vector.tensor_tensor(x4[i][:], x4[i][:], u4[i][:], Alu.add)
        w4 = x4

        # ---------- transposes: AT [NC_][128, M], wT [NC_][128, BPC] ----------
        def transpose_rows(src_tiles, n_src, j, width, tag):
            """src_tiles: list of [128, NF]-like tiles; produce the j-th
            128-col block transposed: out [128, n_src*128] sbuf tile."""
            out_t = pool.tile([128, n_src * 128], F32, tag=tag)
            for h in range((n_src * 128 + 511) // 512):
                ps = psum.tile([128, min(512, n_src*128 - h*512)], F32, tag="ps")
                for q in range(ps.shape[1] // 128):
                    s = h * 4 + q
                    nc.tensor.transpose(ps[:, q*128:(q+1)*128],
                                        src_tiles[s][:, j*128:(j+1)*128],
                                        ident[:])
                nc.vector.tensor_copy(out_t[:, h*512:h*512+ps.shape[1]], ps[:])
            return out_t

        AT = [transpose_rows(A8, MC, j, NF, f"AT{j}") for j in range(NC_)]
        wT = [transpose_rows(w4, BC, j, NF, f"shT{j}") for j in range(NC_)]

        # ---------- Q (fp32 matmuls) -> Qr (f32r) ----------
        Qr = []
        for m in range(MC):
            qt = pool.tile([128, M], LOOP_DT, tag=f"Q{m}")
            for h in range(2):
                ps = psum.tile([128, 512], F32, tag="ps")
                for j in range(NC_):
                    nc.tensor.matmul(ps[:], AT[j][:, m*128:(m+1)*128],
                                     AT[j][:, h*512:(h+1)*512],
                                     start=(j == 0), stop=(j == NC_ - 1))
                nc.vector.tensor_copy(qt[:, h*512:(h+1)*512], ps[:])
            Qr.append(qt)

        # ---------- c -> y init (fp32) ----------
        y8, lam8 = [], []
        for m in range(MC):
            ps = psum.tile([128, BPC], F32, tag="ps")
            for j in range(NC_):
                nc.tensor.matmul(ps[:], AT[j][:, m*128:(m+1)*128], wT[j][:],
                                 start=(j == 0), stop=(j == NC_ - 1))
            ty = pool.tile([128, BPC], F32, tag=f"y{m}")
            nc.vector.tensor_scalar(out=ty[:], in0=ps[:], scalar1=bc8[m][:],
                                    scalar2=STEP, op0=Alu.subtract, op1=Alu.mult)
            y8.append(ty)
            tl = pool.tile([128, BPC], F32, tag=f"lam{m}")
            nc.vector.memset(tl[:], 0.0)
            lam8.append(tl)

        # ---------- PGD delta loop (shared for projx / proju) ----------
        def make_delta(m, active):
            """d[m] = relu(y[m])[*active] - lam[m] (f32r), lam += d, y += d."""
            td = dpool.tile([128, BPC], LOOP_DT, tag=f"d{m}")
            if active is None:
                nc.vector.scalar_tensor_tensor(
                    out=td[:], in0=y8[m][:], scalar=0.0, in1=lam8[m][:],
                    op0=Alu.max, op1=Alu.subtract)
            else:
                tmp = dpool.tile([128, BPC], F32, tag="tmp")
                nc.vector.scalar_tensor_tensor(
                    out=tmp[:], in0=y8[m][:], scalar=0.0,
                    in1=active[m][:], op0=Alu.max, op1=Alu.mult)
                nc.vector.tensor_tensor(td[:], tmp[:], lam8[m][:],
                                        Alu.subtract)
            nc.vector.tensor_tensor(lam8[m][:], lam8[m][:], td[:], Alu.add)
            nc.vector.tensor_tensor(y8[m][:], y8[m][:], td[:], Alu.add)
            return td

        def pgd_loop(iters, active=None):
            for it in range(iters - 1):
                dnew = [make_delta(m, active) for m in range(MC)]
                for m in range(MC):
                    ps = psum.tile([128, BPC], F32, tag="ps")
                    for k in range(MC):
                        nc.tensor.matmul(ps[:], Qr[k][:, m*128:(m+1)*128],
                                         dnew[k][:],
                                         start=(k == 0), stop=(k == MC - 1))
                    nc.vector.scalar_tensor_tensor(
                        out=y8[m][:], in0=ps[:], scalar=-STEP, in1=y8[m][:],
                        op0=Alu.mult, op1=Alu.add)

        # projx: 50 iterations
        pgd_loop(PROJ_ITERS)
        lamx = []
        for m in range(MC):
            t = pool.tile([128, BPC], F32, tag=f"lfin{m}")
            nc.vector.tensor_scalar_max(t[:], y8[m][:], 0.0)
            lamx.append(t)

        # ---------- z = w - lamx@A (natural layout) ----------
        z4 = []
        for i in range(BC):
            ps = psum.tile([128, NF], F32, tag="ps")
            for m in range(MC):
                nc.tensor.matmul(ps[:], lamx[m][:, i*128:(i+1)*128], A8[m][:],
                                 start=(m == 0), stop=(m == MC - 1))
            tz = pool.tile([128, NF], F32, tag=f"z{i}")
            nc.vector.tensor_tensor(tz[:], w4[i][:], ps[:], Alu.subtract)
            z4.append(tz)

        # zT reuses the wT slots (same tag), uT gets its own
        zT = [transpose_rows(z4, BC, j, NF, f"shT{j}") for j in range(NC_)]
        uT = [transpose_rows(u4, BC, j, NF, f"x{j}") for j in range(NC_)]  # w slots

        # ---------- active mask + proju y init ----------
        activeT = []
        for m in range(MC):
            btol = pool.tile([128, 1], F32, tag=f"btol{m}")
            nc.vector.tensor_scalar_sub(btol[:], bc8[m][:], TOL)
            ps = psum.tile([128, BPC], F32, tag="ps")
            for j in range(NC_):
                nc.tensor.matmul(ps[:], AT[j][:, m*128:(m+1)*128], zT[j][:],
                                 start=(j == 0), stop=(j == NC_ - 1))
            ta = pool.tile([128, BPC], BF16, tag=f"act{m}")
            nc.vector.tensor_scalar(out=ta[:], in0=ps[:], scalar1=btol[:],
                                    scalar2=0.0, op0=Alu.subtract, op1=Alu.is_ge)
            activeT.append(ta)
            ps2 = psum.tile([128, BPC], F32, tag="ps")
            for j in range(NC_):
                nc.tensor.matmul(ps2[:], AT[j][:, m*128:(m+1)*128], uT[j][:],
                                 start=(j == 0), stop=(j == NC_ - 1))
            nc.vector.scalar_tensor_tensor(
                out=y8[m][:], in0=ps2[:], scalar=STEP, in1=ta[:],
                op0=Alu.mult, op1=Alu.mult)
            nc.vector.memset(lam8[m][:], 0.0)

        # proju: 10 iterations
        pgd_loop(PROJU_ITERS, active=activeT)
        lamu = []
        for m in range(MC):
            t = pool.tile([128, BPC], F32, tag=f"lfin{m}")  # reuse lamx slots
            nc.vector.scalar_tensor_tensor(
                out=t[:], in0=y8[m][:], scalar=0.0, in1=activeT[m][:],
                op0=Alu.max, op1=Alu.mult)
            lamu.append(t)

        # ---------- out = u - lamu@A ----------
        for i in range(BC):
            ps = psum.tile([128, NF], F32, tag="ps")
            for m in range(MC):
                nc.tensor.matmul(ps[:], lamu[m][:, i*128:(i+1)*128], A8[m][:],
                                 start=(m == 0), stop=(m == MC - 1))
            to = pool.tile([128, NF], F32, tag=f"z{i}")  # z slots are dead
            nc.vector.tensor_tensor(to[:], u4[i][:], ps[:], Alu.subtract)
            nc.sync.dma_start(od[i*128:(i+1)*128, :], to[:])

    nc.compile()
    return nc


def kernel(x, u, A, b):
    x = np.ascontiguousarray(x, dtype=np.float32)
    u = np.ascontiguousarray(u, dtype=np.float32)
    A = np.ascontiguousarray(A, dtype=np.float32)
    b2 = np.ascontiguousarray(b, dtype=np.float32).reshape(M, 1)

    if "nc" not in _cache:
        _cache["nc"] = _build()
    nc = _cache["nc"]

    in_maps = []
    for i in range(NCORES):
        sl = slice(i * BPC, (i + 1) * BPC)
        in_maps.append({"x": x[sl], "u": u[sl], "A": A, "b": b2})
    res = run_bass_kernel_spmd(nc, in_maps, list(range(NCORES)))
    out = np.concatenate([res.results[i]["out"] for i in range(NCORES)], axis=0)
    return out.astype(np.float32)



# revision 3
# speedup vs baseline: 7.6862x; 7.6862x over previous
"""ConvexPolytopeManifold expmap kernel for 8 Trainium2 NeuronCores.

Algorithm (matches reference.py):
    Q = A @ A.T
    z = projx(x+u):  50 its of lam <- relu(lam - step*(lam@Q - c)), c = (x+u)@A.T - b
    out = proju(z,u): active = (z@A.T >= b - tol); masked = (u@A.T)*active
                      10 its of lam <- relu(lam - step*(lam@Q - masked))*active
                      out = u - lam@A

Numerics: the PGD loops run in *delta form* — y (pre-relu state) and lam are
kept in fp32 in SBUF; only the per-iteration increment d = relu(y)-lam goes
through the PE at float32r (round-to-nearest-11-bit-mantissa operands, fp32
accumulate), and its bf16-class error is damped by step=0.01:
    y <- y + d - step*(Q_r @ d)
The d tile is written *as f32r* (rounds on write), so the PE, the lam
accumulation and the y accumulation all consume the identical value — the
recursion stays exactly consistent with lam = sum(d).
All one-shot matmuls on the mask-critical path (c, z, z@A.T, u@A.T, out)
run in plain fp32 (4 cyc/row) for exactness.

Perf structure: the PGD iteration is a hardware For_i loop (body emitted
once), which keeps the program ~500 instructions instead of ~6000 for a
full unroll — the NEFF is what gets re-shipped/loaded per call under the
axon PJRT path, so program size dominates the measured per-call time.
Inside the body the element-wise work is split across engines: DVE does
d = relu(y)-lam and the post-matmul y update, Pool (gpsimd) does the
lam += d and w = y + d accumulations, PE streams the 64 accumulating
matmuls k-major so each Q row-block's matmuls only wait on d[k].

Sharding: data-parallel over batch B=4096 -> 8 cores x 512 rows; A, b, Q
replicated per core. No cross-core communication.
"""
import numpy as np
from contextlib import ExitStack

import concourse.bass as bass
import concourse.tile as tile
from concourse import bacc, mybir
from concourse.bass_utils import run_bass_kernel_spmd
from concourse.masks import make_identity

dt = mybir.dt
F32, F32R, BF16 = dt.float32, dt.float32r, dt.bfloat16
Alu = mybir.AluOpType

B, NF, M = 4096, 512, 1024      # batch, n features, m constraints
NCORES = 8
BPC = B // NCORES               # 512 batch rows per core
PROJ_ITERS, PROJU_ITERS = 50, 10
STEP, TOL = 0.01, 1e-5
MC = M // 128                   # 8 m-chunks
NC_ = NF // 128                 # 4 n-chunks
BC = BPC // 128                 # 4 batch-chunks

_cache = {}


def _build():
    nc = bacc.Bacc("TRN2", target_bir_lowering=False, debug=False,
                   num_devices=NCORES)
    xd = nc.dram_tensor("x", [BPC, NF], F32, kind="ExternalInput").ap()
    ud = nc.dram_tensor("u", [BPC, NF], F32, kind="ExternalInput").ap()
    Ad = nc.dram_tensor("A", [M, NF], F32, kind="ExternalInput").ap()
    bd = nc.dram_tensor("b", [M, 1], F32, kind="ExternalInput").ap()
    od = nc.dram_tensor("out", [BPC, NF], F32, kind="ExternalOutput").ap()

    with tile.TileContext(nc) as tc, ExitStack() as ctx:
        pool = ctx.enter_context(tc.tile_pool(name="main", bufs=1))
        psum = ctx.enter_context(tc.tile_pool(name="ps", bufs=1, space="PSUM"))

        # 8 persistent PSUM banks [128, 512] f32 — exactly fills PSUM.
        ps8 = [psum.tile([128, BPC], F32, tag=f"ps{m}", name=f"ps{m}")
               for m in range(MC)]

        # ---------- loads ----------
        x4, u4, A8, bc8 = [], [], [], []
        for i in range(BC):
            t = pool.tile([128, NF], F32, tag=f"x{i}")
            nc.sync.dma_start(t[:], xd[i*128:(i+1)*128, :]); x4.append(t)
            t = pool.tile([128, NF], F32, tag=f"u{i}")
            nc.sync.dma_start(t[:], ud[i*128:(i+1)*128, :]); u4.append(t)
        for m in range(MC):
            t = pool.tile([128, NF], F32, tag=f"A{m}")
            nc.sync.dma_start(t[:], Ad[m*128:(m+1)*128, :]); A8.append(t)
            t = pool.tile([128, 1], F32, tag=f"b{m}")
            nc.sync.dma_start(t[:], bd[m*128:(m+1)*128, :]); bc8.append(t)

        ident = pool.tile([128, 128], F32, tag="ident")
        make_identity(nc, ident[:])

        # w = x + u  (into x tiles)
        for i in range(BC):
            nc.vector.tensor_tensor(x4[i][:], x4[i][:], u4[i][:], Alu.add)
        w4 = x4

        # ---------- transposes: AT [NC_][128, M], wT [NC_][128, BPC] ----------
        _ps_rot = [0]

        def transpose_rows(src_tiles, n_src, j, tag):
            """Produce the j-th 128-col block of src transposed:
            out [128, n_src*128] sbuf tile."""
            out_t = pool.tile([128, n_src * 128], F32, tag=tag)
            for h in range((n_src * 128 + 511) // 512):
                wdt = min(512, n_src * 128 - h * 512)
                ps = ps8[_ps_rot[0] % MC]; _ps_rot[0] += 1
                for q in range(wdt // 128):
                    s = h * 4 + q
                    nc.tensor.transpose(ps[:, q*128:(q+1)*128],
                                        src_tiles[s][:, j*128:(j+1)*128],
                                        ident[:])
                nc.vector.tensor_copy(out_t[:, h*512:h*512+wdt], ps[:, :wdt])
            return out_t

        AT = [transpose_rows(A8, MC, j, f"AT{j}") for j in range(NC_)]
        wT = [transpose_rows(w4, BC, j, f"shT{j}") for j in range(NC_)]

        # ---------- Q (fp32 matmuls) -> Qr (f32r) ----------
        Qr = []
        for m in range(MC):
            qt = pool.tile([128, M], F32R, tag=f"Q{m}")
            for h in range(2):
                ps = ps8[_ps_rot[0] % MC]; _ps_rot[0] += 1
                for j in range(NC_):
                    nc.tensor.matmul(ps[:], AT[j][:, m*128:(m+1)*128],
                                     AT[j][:, h*512:(h+1)*512],
                                     start=(j == 0), stop=(j == NC_ - 1))
                nc.vector.tensor_copy(qt[:, h*512:(h+1)*512], ps[:])
            Qr.append(qt)

        # ---------- c -> y init (fp32); state tiles ----------
        y8, lam8, w8, d8, lfin8 = [], [], [], [], []
        for m in range(MC):
            ps = ps8[m]
            for j in range(NC_):
                nc.tensor.matmul(ps[:], AT[j][:, m*128:(m+1)*128], wT[j][:],
                                 start=(j == 0), stop=(j == NC_ - 1))
            ty = pool.tile([128, BPC], F32, tag=f"y{m}")
            nc.vector.tensor_scalar(out=ty[:], in0=ps[:], scalar1=bc8[m][:],
                                    scalar2=STEP, op0=Alu.subtract, op1=Alu.mult)
            y8.append(ty)
            tl = pool.tile([128, BPC], F32, tag=f"lam{m}")
            nc.vector.memset(tl[:], 0.0)
            lam8.append(tl)
            w8.append(pool.tile([128, BPC], F32, tag=f"w{m}", name=f"w{m}"))
            d8.append(pool.tile([128, BPC], F32R, tag=f"d{m}", name=f"d{m}"))
            lfin8.append(pool.tile([128, BPC], F32, tag=f"lfin{m}", name=f"lfin{m}"))

        # ---------- PGD iteration body (shared for projx / proju) ----------
        def pgd_body(active):
            # d[k] = relu(y[k])[*active] - lam[k]  (f32r), then the 8
            # accumulating matmul batches for Q row-block k — PE only
            # waits on d[k], so it starts ~one DVE op into the iteration.
            for k in range(MC):
                if active is None:
                    nc.vector.scalar_tensor_tensor(
                        out=d8[k][:], in0=y8[k][:], scalar=0.0,
                        in1=lam8[k][:], op0=Alu.max, op1=Alu.subtract)
                else:
                    tmp = lfin8[k]
                    nc.vector.scalar_tensor_tensor(
                        out=tmp[:], in0=y8[k][:], scalar=0.0,
                        in1=active[k][:], op0=Alu.max, op1=Alu.mult)
                    nc.vector.tensor_tensor(d8[k][:], tmp[:], lam8[k][:],
                                            Alu.subtract)
                for m in range(MC):
                    nc.tensor.matmul(ps8[m][:], Qr[k][:, m*128:(m+1)*128],
                                     d8[k][:],
                                     start=(k == 0), stop=(k == MC - 1))
                # state accumulations off the critical path -> Pool engine
                nc.gpsimd.tensor_tensor(lam8[k][:], lam8[k][:], d8[k][:],
                                        Alu.add)
                nc.gpsimd.tensor_tensor(w8[k][:], y8[k][:], d8[k][:], Alu.add)
            for m in range(MC):
                nc.vector.scalar_tensor_tensor(
                    out=y8[m][:], in0=ps8[m][:], scalar=-STEP, in1=w8[m][:],
                    op0=Alu.mult, op1=Alu.add)

        # projx: 50 iterations == 49 in-loop y updates + final relu
        with tc.For_i(0, PROJ_ITERS - 1):
            pgd_body(None)
        for m in range(MC):
            nc.vector.tensor_scalar_max(lfin8[m][:], y8[m][:], 0.0)
        lamx = lfin8

        # ---------- z = w - lamx@A (natural layout) ----------
        z4 = []
        for i in range(BC):
            ps = ps8[i]
            for m in range(MC):
                nc.tensor.matmul(ps[:], lamx[m][:, i*128:(i+1)*128], A8[m][:],
                                 start=(m == 0), stop=(m == MC - 1))
            tz = pool.tile([128, NF], F32, tag=f"z{i}")
            nc.vector.tensor_tensor(tz[:], w4[i][:], ps[:], Alu.subtract)
            z4.append(tz)

        # zT reuses the wT slots (same tag), uT the w (=x) slots
        zT = [transpose_rows(z4, BC, j, f"shT{j}") for j in range(NC_)]
        uT = [transpose_rows(u4, BC, j, f"x{j}") for j in range(NC_)]

        # ---------- active mask + proju y init ----------
        activeT = []
        for m in range(MC):
            btol = pool.tile([128, 1], F32, tag=f"btol{m}")
            nc.vector.tensor_scalar_sub(btol[:], bc8[m][:], TOL)
            ps = ps8[(2*m) % MC]
            for j in range(NC_):
                nc.tensor.matmul(ps[:], AT[j][:, m*128:(m+1)*128], zT[j][:],
                                 start=(j == 0), stop=(j == NC_ - 1))
            ta = pool.tile([128, BPC], BF16, tag=f"act{m}")
            nc.vector.tensor_scalar(out=ta[:], in0=ps[:], scalar1=btol[:],
                                    scalar2=0.0, op0=Alu.subtract, op1=Alu.is_ge)
            activeT.append(ta)
            ps2 = ps8[(2*m + 1) % MC]
            for j in range(NC_):
                nc.tensor.matmul(ps2[:], AT[j][:, m*128:(m+1)*128], uT[j][:],
                                 start=(j == 0), stop=(j == NC_ - 1))
            nc.vector.scalar_tensor_tensor(
                out=y8[m][:], in0=ps2[:], scalar=STEP, in1=ta[:],
                op0=Alu.mult, op1=Alu.mult)
            nc.vector.memset(lam8[m][:], 0.0)

        # proju: 10 iterations == 9 in-loop + final masked relu
        with tc.For_i(0, PROJU_ITERS - 1):
            pgd_body(activeT)
        for m in range(MC):
            nc.vector.scalar_tensor_tensor(
                out=lfin8[m][:], in0=y8[m][:], scalar=0.0, in1=activeT[m][:],
                op0=Alu.max, op1=Alu.mult)
        lamu = lfin8

        # ---------- out = u - lamu@A ----------
        for i in range(BC):
            ps = ps8[i]
            for m in range(MC):
                nc.tensor.matmul(ps[:], lamu[m][:, i*128:(i+1)*128], A8[m][:],
                                 start=(m == 0), stop=(m == MC - 1))
            to = pool.tile([128, NF], F32, tag=f"z{i}")  # z slots are dead
            nc.vector.tensor_tensor(to[:], u4[i][:], ps[:], Alu.subtract)
            nc.sync.dma_start(od[i*128:(i+1)*128, :], to[:])

    nc.compile()
    return nc


def kernel(x, u, A, b):
    x = np.ascontiguousarray(x, dtype=np.float32)
    u = np.ascontiguousarray(u, dtype=np.float32)
    A = np.ascontiguousarray(A, dtype=np.float32)
    b2 = np.ascontiguousarray(b, dtype=np.float32).reshape(M, 1)

    if "nc" not in _cache:
        _cache["nc"] = _build()
    nc = _cache["nc"]

    in_maps = []
    for i in range(NCORES):
        sl = slice(i * BPC, (i + 1) * BPC)
        in_maps.append({"x": x[sl], "u": u[sl], "A": A, "b": b2})
    res = run_bass_kernel_spmd(nc, in_maps, list(range(NCORES)))
    out = np.concatenate([res.results[i]["out"] for i in range(NCORES)], axis=0)
    return out.astype(np.float32)


# revision 6
# speedup vs baseline: 12.9432x; 1.6839x over previous
"""ConvexPolytopeManifold expmap kernel for 8 Trainium2 NeuronCores.

Algorithm (matches reference.py):
    Q = A @ A.T
    z = projx(x+u):  50 its of lam <- relu(lam - step*(lam@Q - c)), c = (x+u)@A.T - b
    out = proju(z,u): active = (z@A.T >= b - tol); masked = (u@A.T)*active
                      10 its of lam <- relu(lam - step*(lam@Q - masked))*active
                      out = u - lam@A

Numerics: the PGD loops run in *delta form* — y (pre-relu state) and lam are
kept in fp32 in SBUF; only the per-iteration increment d = relu(y)-lam goes
through the PE at float32r (round-to-nearest-11-bit-mantissa operands, fp32
accumulate), and its bf16-class error is damped by step=0.01:
    y <- y + d - step*(Q_r @ d)
The d tile is written *as f32r* (rounds on write), so the PE, the lam
accumulation and the y accumulation all consume the identical value — the
recursion stays exactly consistent with lam = sum(d).
All one-shot matmuls on the mask-critical path (c, z, z@A.T, u@A.T, out)
run in plain fp32 (4 cyc/row) for exactness.

Perf structure: the PGD iteration is a hardware For_i loop (body emitted
once), which keeps the program ~500 instructions instead of ~6000 for a
full unroll — the NEFF is what gets re-shipped/loaded per call under the
axon PJRT path, so program size dominates the measured per-call time.
Inside the body the element-wise work is split across engines: DVE does
d = relu(y)-lam and the post-matmul y update, Pool (gpsimd) does the
lam += d and w = y + d accumulations, PE streams the 64 accumulating
matmuls k-major so each Q row-block's matmuls only wait on d[k].

Sharding: data-parallel over batch B=4096 -> 8 cores x 512 rows; A, b, Q
replicated per core. No cross-core communication.
"""
import os
import tempfile

import numpy as np
from contextlib import ExitStack

# Persistent XLA compilation cache: run_bass_kernel_spmd builds a fresh
# jax.jit per call, so without this every kernel() call re-runs the
# XLA+walrus compile (~160ms). With it, warm calls hit the cache.
try:
    import jax

    _cache_dir = os.path.join(
        os.path.expanduser("~") if os.access(os.path.expanduser("~"), os.W_OK)
        else tempfile.gettempdir(), ".jax_comp_cache")
    os.makedirs(_cache_dir, exist_ok=True)
    jax.config.update("jax_compilation_cache_dir", _cache_dir)
    jax.config.update("jax_persistent_cache_min_compile_time_secs", 0.0)
    jax.config.update("jax_persistent_cache_min_entry_size_bytes", 0)
except Exception:
    pass

import concourse.bass as bass
import concourse.tile as tile
from concourse import bacc, mybir
from concourse.bass_utils import run_bass_kernel_spmd
from concourse.masks import make_identity

dt = mybir.dt
F32, F32R, BF16 = dt.float32, dt.float32r, dt.bfloat16
Alu = mybir.AluOpType

B, NF, M = 4096, 512, 1024      # batch, n features, m constraints
NCORES = 8
BPC = B // NCORES               # 512 batch rows per core
PROJ_ITERS, PROJU_ITERS = 50, 10
STEP, TOL = 0.01, 1e-5
MC = M // 128                   # 8 m-chunks
NC_ = NF // 128                 # 4 n-chunks
BC = BPC // 128                 # 4 batch-chunks

_cache = {}
_REPS = 1   # bench hook: >1 wraps the whole per-core program in For_i


def _build():
    import contextlib
    nc = bacc.Bacc("TRN2", target_bir_lowering=False, debug=False,
                   num_devices=NCORES)
    xd = nc.dram_tensor("x", [BPC, NF], F32, kind="ExternalInput").ap()
    ud = nc.dram_tensor("u", [BPC, NF], F32, kind="ExternalInput").ap()
    Ad = nc.dram_tensor("A", [M, NF], F32, kind="ExternalInput").ap()
    bd = nc.dram_tensor("b", [M, 1], F32, kind="ExternalInput").ap()
    od = nc.dram_tensor("out", [BPC, NF], F32, kind="ExternalOutput").ap()

    with tile.TileContext(nc) as tc, ExitStack() as ctx:
        pool = ctx.enter_context(tc.tile_pool(name="main", bufs=1))
        psum = ctx.enter_context(tc.tile_pool(name="ps", bufs=1, space="PSUM"))

        rep_loop = tc.For_i(0, _REPS) if _REPS > 1 else contextlib.nullcontext()
        ctx.enter_context(rep_loop)

        # 8 persistent PSUM banks [128, 512] f32 — exactly fills PSUM.
        ps8 = [psum.tile([128, BPC], F32, tag=f"ps{m}", name=f"ps{m}")
               for m in range(MC)]

        # ---------- loads ----------
        x4, u4, A8, bc8 = [], [], [], []
        for i in range(BC):
            t = pool.tile([128, NF], F32, tag=f"x{i}")
            nc.sync.dma_start(t[:], xd[i*128:(i+1)*128, :]); x4.append(t)
            t = pool.tile([128, NF], F32, tag=f"u{i}")
            nc.sync.dma_start(t[:], ud[i*128:(i+1)*128, :]); u4.append(t)
        for m in range(MC):
            t = pool.tile([128, NF], F32, tag=f"A{m}")
            nc.sync.dma_start(t[:], Ad[m*128:(m+1)*128, :]); A8.append(t)
            t = pool.tile([128, 1], F32, tag=f"b{m}")
            nc.sync.dma_start(t[:], bd[m*128:(m+1)*128, :]); bc8.append(t)

        ident = pool.tile([128, 128], F32, tag="ident")
        make_identity(nc, ident[:])

        # w = x + u  (into x tiles)
        for i in range(BC):
            nc.vector.tensor_tensor(x4[i][:], x4[i][:], u4[i][:], Alu.add)
        w4 = x4

        # ---------- transposes: AT [NC_][128, M], wT [NC_][128, BPC] ----------
        _ps_rot = [0]

        def transpose_rows(src_tiles, n_src, j, tag):
            """Produce the j-th 128-col block of src transposed:
            out [128, n_src*128] sbuf tile."""
            out_t = pool.tile([128, n_src * 128], F32, tag=tag)
            for h in range((n_src * 128 + 511) // 512):
                wdt = min(512, n_src * 128 - h * 512)
                ps = ps8[_ps_rot[0] % MC]; _ps_rot[0] += 1
                for q in range(wdt // 128):
                    s = h * 4 + q
                    nc.tensor.transpose(ps[:, q*128:(q+1)*128],
                                        src_tiles[s][:, j*128:(j+1)*128],
                                        ident[:])
                nc.vector.tensor_copy(out_t[:, h*512:h*512+wdt], ps[:, :wdt])
            return out_t

        AT = [transpose_rows(A8, MC, j, f"AT{j}") for j in range(NC_)]
        wT = [transpose_rows(w4, BC, j, f"shT{j}") for j in range(NC_)]

        # ---------- Q (fp32 matmuls) -> Qr (f32r) ----------
        Qr = []
        for m in range(MC):
            qt = pool.tile([128, M], F32R, tag=f"Q{m}")
            for h in range(2):
                ps = ps8[_ps_rot[0] % MC]; _ps_rot[0] += 1
                for j in range(NC_):
                    nc.tensor.matmul(ps[:], AT[j][:, m*128:(m+1)*128],
                                     AT[j][:, h*512:(h+1)*512],
                                     start=(j == 0), stop=(j == NC_ - 1))
                nc.vector.tensor_copy(qt[:, h*512:(h+1)*512], ps[:])
            Qr.append(qt)

        # ---------- c -> y init (fp32); state tiles ----------
        y8, lam8, w8, d8, lfin8 = [], [], [], [], []
        for m in range(MC):
            ps = ps8[m]
            for j in range(NC_):
                nc.tensor.matmul(ps[:], AT[j][:, m*128:(m+1)*128], wT[j][:],
                                 start=(j == 0), stop=(j == NC_ - 1))
            ty = pool.tile([128, BPC], F32, tag=f"y{m}")
            nc.vector.tensor_scalar(out=ty[:], in0=ps[:], scalar1=bc8[m][:],
                                    scalar2=STEP, op0=Alu.subtract, op1=Alu.mult)
            y8.append(ty)
            tl = pool.tile([128, BPC], F32, tag=f"lam{m}")
            nc.vector.memset(tl[:], 0.0)
            lam8.append(tl)
            w8.append(pool.tile([128, BPC], F32, tag=f"w{m}", name=f"w{m}"))
            d8.append(pool.tile([128, BPC], F32R, tag=f"d{m}", name=f"d{m}"))
            lfin8.append(pool.tile([128, BPC], F32, tag=f"lfin{m}", name=f"lfin{m}"))

        # ---------- PGD iteration body (shared for projx / proju) ----------
        def pgd_body(active):
            # d[k] = relu(y[k])[*active] - lam[k]  (f32r), then the 8
            # accumulating matmul batches for Q row-block k — PE only
            # waits on d[k], so it starts ~one DVE op into the iteration.
            for k in range(MC):
                if active is None:
                    nc.vector.scalar_tensor_tensor(
                        out=d8[k][:], in0=y8[k][:], scalar=0.0,
                        in1=lam8[k][:], op0=Alu.max, op1=Alu.subtract)
                else:
                    tmp = lfin8[k]
                    nc.vector.scalar_tensor_tensor(
                        out=tmp[:], in0=y8[k][:], scalar=0.0,
                        in1=active[k][:], op0=Alu.max, op1=Alu.mult)
                    nc.vector.tensor_tensor(d8[k][:], tmp[:], lam8[k][:],
                                            Alu.subtract)
                for m in range(MC):
                    nc.tensor.matmul(ps8[m][:], Qr[k][:, m*128:(m+1)*128],
                                     d8[k][:],
                                     start=(k == 0), stop=(k == MC - 1))
                # state accumulations off the critical path -> Pool engine
                nc.gpsimd.tensor_tensor(lam8[k][:], lam8[k][:], d8[k][:],
                                        Alu.add)
                nc.gpsimd.tensor_tensor(w8[k][:], y8[k][:], d8[k][:], Alu.add)
            for m in range(MC):
                nc.vector.scalar_tensor_tensor(
                    out=y8[m][:], in0=ps8[m][:], scalar=-STEP, in1=w8[m][:],
                    op0=Alu.mult, op1=Alu.add)

        # projx: 50 iterations == 49 in-loop y updates + final relu
        with tc.For_i(0, PROJ_ITERS - 1):
            pgd_body(None)
        for m in range(MC):
            nc.vector.tensor_scalar_max(lfin8[m][:], y8[m][:], 0.0)
        lamx = lfin8

        # ---------- z = w - lamx@A (natural layout) ----------
        z4 = []
        for i in range(BC):
            ps = ps8[i]
            for m in range(MC):
                nc.tensor.matmul(ps[:], lamx[m][:, i*128:(i+1)*128], A8[m][:],
                                 start=(m == 0), stop=(m == MC - 1))
            tz = pool.tile([128, NF], F32, tag=f"z{i}")
            nc.vector.tensor_tensor(tz[:], w4[i][:], ps[:], Alu.subtract)
            z4.append(tz)

        # zT reuses the wT slots (same tag), uT the w (=x) slots
        zT = [transpose_rows(z4, BC, j, f"shT{j}") for j in range(NC_)]
        uT = [transpose_rows(u4, BC, j, f"x{j}") for j in range(NC_)]

        # ---------- active mask + proju y init ----------
        activeT = []
        for m in range(MC):
            btol = pool.tile([128, 1], F32, tag=f"btol{m}")
            nc.vector.tensor_scalar_sub(btol[:], bc8[m][:], TOL)
            ps = ps8[(2*m) % MC]
            for j in range(NC_):
                nc.tensor.matmul(ps[:], AT[j][:, m*128:(m+1)*128], zT[j][:],
                                 start=(j == 0), stop=(j == NC_ - 1))
            ta = pool.tile([128, BPC], BF16, tag=f"act{m}")
            nc.vector.tensor_scalar(out=ta[:], in0=ps[:], scalar1=btol[:],
                                    scalar2=0.0, op0=Alu.subtract, op1=Alu.is_ge)
            activeT.append(ta)
            ps2 = ps8[(2*m + 1) % MC]
            for j in range(NC_):
                nc.tensor.matmul(ps2[:], AT[j][:, m*128:(m+1)*128], uT[j][:],
                                 start=(j == 0), stop=(j == NC_ - 1))
            nc.vector.scalar_tensor_tensor(
                out=y8[m][:], in0=ps2[:], scalar=STEP, in1=ta[:],
                op0=Alu.mult, op1=Alu.mult)
            nc.vector.memset(lam8[m][:], 0.0)

        # proju: 10 iterations == 9 in-loop + final masked relu
        with tc.For_i(0, PROJU_ITERS - 1):
            pgd_body(activeT)
        for m in range(MC):
            nc.vector.scalar_tensor_tensor(
                out=lfin8[m][:], in0=y8[m][:], scalar=0.0, in1=activeT[m][:],
                op0=Alu.max, op1=Alu.mult)
        lamu = lfin8

        # ---------- out = u - lamu@A ----------
        for i in range(BC):
            ps = ps8[i]
            for m in range(MC):
                nc.tensor.matmul(ps[:], lamu[m][:, i*128:(i+1)*128], A8[m][:],
                                 start=(m == 0), stop=(m == MC - 1))
            to = pool.tile([128, NF], F32, tag=f"z{i}")  # z slots are dead
            nc.vector.tensor_tensor(to[:], u4[i][:], ps[:], Alu.subtract)
            nc.sync.dma_start(od[i*128:(i+1)*128, :], to[:])

    nc.compile()
    return nc


def kernel(x, u, A, b):
    x = np.ascontiguousarray(x, dtype=np.float32)
    u = np.ascontiguousarray(u, dtype=np.float32)
    A = np.ascontiguousarray(A, dtype=np.float32)
    b2 = np.ascontiguousarray(b, dtype=np.float32).reshape(M, 1)

    if "nc" not in _cache:
        _cache["nc"] = _build()
    nc = _cache["nc"]

    in_maps = []
    for i in range(NCORES):
        sl = slice(i * BPC, (i + 1) * BPC)
        in_maps.append({"x": x[sl], "u": u[sl], "A": A, "b": b2})
    res = run_bass_kernel_spmd(nc, in_maps, list(range(NCORES)))
    out = np.concatenate([res.results[i]["out"] for i in range(NCORES)], axis=0)
    return out.astype(np.float32)


# revision 9
# speedup vs baseline: 37.3698x; 2.8872x over previous
"""ConvexPolytopeManifold expmap kernel for 8 Trainium2 NeuronCores.

Algorithm (matches reference.py):
    Q = A @ A.T
    z = projx(x+u):  50 its of lam <- relu(lam - step*(lam@Q - c)), c = (x+u)@A.T - b
    out = proju(z,u): active = (z@A.T >= b - tol); masked = (u@A.T)*active
                      10 its of lam <- relu(lam - step*(lam@Q - masked))*active
                      out = u - lam@A

Numerics: the PGD loops run in *delta form* — y (pre-relu state) and lam are
kept in fp32 in SBUF; only the per-iteration increment d = relu(y)-lam goes
through the PE at float32r (round-to-nearest-11-bit-mantissa operands, fp32
accumulate), and its bf16-class error is damped by step=0.01:
    y <- y + d - step*(Q_r @ d)
The d tile is written *as f32r* (rounds on write), so the PE, the lam
accumulation and the y accumulation all consume the identical value — the
recursion stays exactly consistent with lam = sum(d).
All one-shot matmuls on the mask-critical path (c, z, z@A.T, u@A.T, out)
run in plain fp32 (4 cyc/row) for exactness.

Perf structure: the PGD iteration is a hardware For_i loop (body emitted
once), which keeps the program ~500 instructions instead of ~6000 for a
full unroll — the NEFF is what gets re-shipped/loaded per call under the
axon PJRT path, so program size dominates the measured per-call time.
Inside the body the element-wise work is split across engines: DVE does
d = relu(y)-lam and the post-matmul y update, Pool (gpsimd) does the
lam += d and w = y + d accumulations, PE streams the 64 accumulating
matmuls k-major so each Q row-block's matmuls only wait on d[k].

Sharding: data-parallel over batch B=4096 -> 8 cores x 512 rows; A, b, Q
replicated per core. No cross-core communication.
"""
import os
import tempfile

import numpy as np
from contextlib import ExitStack

# Persistent XLA compilation cache: run_bass_kernel_spmd builds a fresh
# jax.jit per call, so without this every kernel() call re-runs the
# XLA+walrus compile (~160ms). With it, warm calls hit the cache.
try:
    import jax

    _cache_dir = os.path.join(
        os.path.expanduser("~") if os.access(os.path.expanduser("~"), os.W_OK)
        else tempfile.gettempdir(), ".jax_comp_cache")
    os.makedirs(_cache_dir, exist_ok=True)
    jax.config.update("jax_compilation_cache_dir", _cache_dir)
    jax.config.update("jax_persistent_cache_min_compile_time_secs", 0.0)
    jax.config.update("jax_persistent_cache_min_entry_size_bytes", 0)
except Exception:
    pass

import concourse.bass as bass
import concourse.tile as tile
from concourse import bacc, mybir
from concourse.bass_utils import run_bass_kernel_spmd
from concourse.masks import make_identity

dt = mybir.dt
F32, F32R, BF16 = dt.float32, dt.float32r, dt.bfloat16
Alu = mybir.AluOpType

B, NF, M = 4096, 512, 1024      # batch, n features, m constraints
NCORES = 8
BPC = B // NCORES               # 512 batch rows per core
PROJ_ITERS, PROJU_ITERS = 50, 10
STEP, TOL = 0.01, 1e-5
MC = M // 128                   # 8 m-chunks
NC_ = NF // 128                 # 4 n-chunks
BC = BPC // 128                 # 4 batch-chunks

_cache = {}
_REPS = 1   # bench hook: >1 wraps the whole per-core program in For_i
LOOP_DT = F32R  # PGD loop matmul dtype (bench hook)


def _build():
    import contextlib
    nc = bacc.Bacc("TRN2", target_bir_lowering=False, debug=False,
                   num_devices=NCORES)
    xd = nc.dram_tensor("x", [BPC, NF], F32, kind="ExternalInput").ap()
    ud = nc.dram_tensor("u", [BPC, NF], F32, kind="ExternalInput").ap()
    Ad = nc.dram_tensor("A", [M, NF], F32, kind="ExternalInput").ap()
    bd = nc.dram_tensor("b", [M, 1], F32, kind="ExternalInput").ap()
    od = nc.dram_tensor("out", [BPC, NF], F32, kind="ExternalOutput").ap()

    with tile.TileContext(nc) as tc, ExitStack() as ctx:
        pool = ctx.enter_context(tc.tile_pool(name="main", bufs=1))
        psum = ctx.enter_context(tc.tile_pool(name="ps", bufs=1, space="PSUM"))

        rep_loop = tc.For_i(0, _REPS) if _REPS > 1 else contextlib.nullcontext()
        ctx.enter_context(rep_loop)

        # 8 persistent PSUM banks [128, 512] f32 — exactly fills PSUM.
        ps8 = [psum.tile([128, BPC], F32, tag=f"ps{m}", name=f"ps{m}")
               for m in range(MC)]

        # ---------- loads ----------
        x4, u4, A8, bc8 = [], [], [], []
        for i in range(BC):
            t = pool.tile([128, NF], F32, tag=f"x{i}")
            nc.sync.dma_start(t[:], xd[i*128:(i+1)*128, :]); x4.append(t)
            t = pool.tile([128, NF], F32, tag=f"u{i}")
            nc.sync.dma_start(t[:], ud[i*128:(i+1)*128, :]); u4.append(t)
        for m in range(MC):
            t = pool.tile([128, NF], F32, tag=f"A{m}")
            nc.sync.dma_start(t[:], Ad[m*128:(m+1)*128, :]); A8.append(t)
            t = pool.tile([128, 1], F32, tag=f"b{m}")
            nc.sync.dma_start(t[:], bd[m*128:(m+1)*128, :]); bc8.append(t)

        ident = pool.tile([128, 128], F32, tag="ident")
        make_identity(nc, ident[:])

        # w = x + u  (into x tiles)
        for i in range(BC):
            nc.vector.tensor_tensor(x4[i][:], x4[i][:], u4[i][:], Alu.add)
        w4 = x4

        # ---------- transposes: AT [NC_][128, M], wT [NC_][128, BPC] ----------
        _ps_rot = [0]

        def transpose_rows(src_tiles, n_src, j, tag):
            """Produce the j-th 128-col block of src transposed:
            out [128, n_src*128] sbuf tile."""
            out_t = pool.tile([128, n_src * 128], F32, tag=tag)
            for h in range((n_src * 128 + 511) // 512):
                wdt = min(512, n_src * 128 - h * 512)
                ps = ps8[_ps_rot[0] % MC]; _ps_rot[0] += 1
                for q in range(wdt // 128):
                    s = h * 4 + q
                    nc.tensor.transpose(ps[:, q*128:(q+1)*128],
                                        src_tiles[s][:, j*128:(j+1)*128],
                                        ident[:])
                nc.vector.tensor_copy(out_t[:, h*512:h*512+wdt], ps[:, :wdt])
            return out_t

        AT = [transpose_rows(A8, MC, j, f"AT{j}") for j in range(NC_)]
        wT = [transpose_rows(w4, BC, j, f"shT{j}") for j in range(NC_)]

        # ---------- Q (fp32 matmuls) -> Qr (f32r) ----------
        Qr = []
        for m in range(MC):
            qt = pool.tile([128, M], LOOP_DT, tag=f"Q{m}")
            for h in range(2):
                ps = ps8[_ps_rot[0] % MC]; _ps_rot[0] += 1
                for j in range(NC_):
                    nc.tensor.matmul(ps[:], AT[j][:, m*128:(m+1)*128],
                                     AT[j][:, h*512:(h+1)*512],
                                     start=(j == 0), stop=(j == NC_ - 1))
                nc.vector.tensor_copy(qt[:, h*512:(h+1)*512], ps[:])
            Qr.append(qt)

        # ---------- c -> y init (fp32); state tiles ----------
        y8, lam8, w8, d8, lfin8 = [], [], [], [], []
        for m in range(MC):
            ps = ps8[m]
            for j in range(NC_):
                nc.tensor.matmul(ps[:], AT[j][:, m*128:(m+1)*128], wT[j][:],
                                 start=(j == 0), stop=(j == NC_ - 1))
            ty = pool.tile([128, BPC], F32, tag=f"y{m}")
            nc.vector.tensor_scalar(out=ty[:], in0=ps[:], scalar1=bc8[m][:],
                                    scalar2=STEP, op0=Alu.subtract, op1=Alu.mult)
            y8.append(ty)
            tl = pool.tile([128, BPC], F32, tag=f"lam{m}")
            nc.vector.memset(tl[:], 0.0)
            lam8.append(tl)
            w8.append(pool.tile([128, BPC], F32, tag=f"w{m}", name=f"w{m}"))
            d8.append(pool.tile([128, BPC], LOOP_DT, tag=f"d{m}", name=f"d{m}"))
            lfin8.append(pool.tile([128, BPC], F32, tag=f"lfin{m}", name=f"lfin{m}"))

        # ---------- PGD iteration body (shared for projx / proju) ----------
        def pgd_body(active):
            # d[k] = relu(y[k])[*active] - lam[k]  (f32r), then the 8
            # accumulating matmul batches for Q row-block k — PE only
            # waits on d[k], so it starts ~one DVE op into the iteration.
            for k in range(MC):
                if active is None:
                    nc.vector.scalar_tensor_tensor(
                        out=d8[k][:], in0=y8[k][:], scalar=0.0,
                        in1=lam8[k][:], op0=Alu.max, op1=Alu.subtract)
                else:
                    tmp = lfin8[k]
                    nc.vector.scalar_tensor_tensor(
                        out=tmp[:], in0=y8[k][:], scalar=0.0,
                        in1=active[k][:], op0=Alu.max, op1=Alu.mult)
                    nc.vector.tensor_tensor(d8[k][:], tmp[:], lam8[k][:],
                                            Alu.subtract)
                for m in range(MC):
                    nc.tensor.matmul(ps8[m][:], Qr[k][:, m*128:(m+1)*128],
                                     d8[k][:],
                                     start=(k == 0), stop=(k == MC - 1))
                # state accumulations off the critical path -> Pool engine
                nc.gpsimd.tensor_tensor(lam8[k][:], lam8[k][:], d8[k][:],
                                        Alu.add)
                nc.gpsimd.tensor_tensor(w8[k][:], y8[k][:], d8[k][:], Alu.add)
            for m in range(MC):
                nc.vector.scalar_tensor_tensor(
                    out=y8[m][:], in0=ps8[m][:], scalar=-STEP, in1=w8[m][:],
                    op0=Alu.mult, op1=Alu.add)

        # projx: 50 iterations == 49 in-loop y updates + final relu
        with tc.For_i(0, PROJ_ITERS - 1):
            pgd_body(None)
        for m in range(MC):
            nc.vector.tensor_scalar_max(lfin8[m][:], y8[m][:], 0.0)
        lamx = lfin8

        # ---------- z = w - lamx@A (natural layout) ----------
        z4 = []
        for i in range(BC):
            ps = ps8[i]
            for m in range(MC):
                nc.tensor.matmul(ps[:], lamx[m][:, i*128:(i+1)*128], A8[m][:],
                                 start=(m == 0), stop=(m == MC - 1))
            tz = pool.tile([128, NF], F32, tag=f"z{i}")
            nc.vector.tensor_tensor(tz[:], w4[i][:], ps[:], Alu.subtract)
            z4.append(tz)

        # zT reuses the wT slots (same tag), uT the w (=x) slots
        zT = [transpose_rows(z4, BC, j, f"shT{j}") for j in range(NC_)]
        uT = [transpose_rows(u4, BC, j, f"x{j}") for j in range(NC_)]

        # ---------- active mask + proju y init ----------
        activeT = []
        for m in range(MC):
            btol = pool.tile([128, 1], F32, tag=f"btol{m}")
            nc.vector.tensor_scalar_sub(btol[:], bc8[m][:], TOL)
            ps = ps8[(2*m) % MC]
            for j in range(NC_):
                nc.tensor.matmul(ps[:], AT[j][:, m*128:(m+1)*128], zT[j][:],
                                 start=(j == 0), stop=(j == NC_ - 1))
            ta = pool.tile([128, BPC], BF16, tag=f"act{m}")
            nc.vector.tensor_scalar(out=ta[:], in0=ps[:], scalar1=btol[:],
                                    scalar2=0.0, op0=Alu.subtract, op1=Alu.is_ge)
            activeT.append(ta)
            ps2 = ps8[(2*m + 1) % MC]
            for j in range(NC_):
                nc.tensor.matmul(ps2[:], AT[j][:, m*128:(m+1)*128], uT[j][:],
                                 start=(j == 0), stop=(j == NC_ - 1))
            nc.vector.scalar_tensor_tensor(
                out=y8[m][:], in0=ps2[:], scalar=STEP, in1=ta[:],
                op0=Alu.mult, op1=Alu.mult)
            nc.vector.memset(lam8[m][:], 0.0)

        # proju: 10 iterations == 9 in-loop + final masked relu
        with tc.For_i(0, PROJU_ITERS - 1):
            pgd_body(activeT)
        for m in range(MC):
            nc.vector.scalar_tensor_tensor(
                out=lfin8[m][:], in0=y8[m][:], scalar=0.0, in1=activeT[m][:],
                op0=Alu.max, op1=Alu.mult)
        lamu = lfin8

        # ---------- out = u - lamu@A ----------
        for i in range(BC):
            ps = ps8[i]
            for m in range(MC):
                nc.tensor.matmul(ps[:], lamu[m][:, i*128:(i+1)*128], A8[m][:],
                                 start=(m == 0), stop=(m == MC - 1))
            to = pool.tile([128, NF], F32, tag=f"z{i}")  # z slots are dead
            nc.vector.tensor_tensor(to[:], u4[i][:], ps[:], Alu.subtract)
            nc.sync.dma_start(od[i*128:(i+1)*128, :], to[:])

    nc.compile()
    return nc


def kernel(x, u, A, b):
    x = np.ascontiguousarray(x, dtype=np.float32)
    u = np.ascontiguousarray(u, dtype=np.float32)
    A = np.ascontiguousarray(A, dtype=np.float32)
    b2 = np.ascontiguousarray(b, dtype=np.float32).reshape(M, 1)

    if "nc" not in _cache:
        nc = _build()
        # run_bass_kernel_spmd re-lowers per call, and the lowering
        # serializes+zstd's the whole BIR each time (~10ms). The program
        # is frozen after _build, so memoize the serialization.
        _bj = nc.to_json_bytes()
        nc.to_json_bytes = lambda: _bj
        _cache["nc"] = nc
    nc = _cache["nc"]

    in_maps = []
    for i in range(NCORES):
        sl = slice(i * BPC, (i + 1) * BPC)
        in_maps.append({"x": x[sl], "u": u[sl], "A": A, "b": b2})
    res = run_bass_kernel_spmd(nc, in_maps, list(range(NCORES)))
    out = np.concatenate([res.results[i]["out"] for i in range(NCORES)], axis=0)
    return out.astype(np.float32)
